# revision 30
# baseline (speedup 1.0000x reference)
"""Trainium2 Bass kernel for nn_MixedHeadsV2 (mixed-head causal attention).

Full inputs in, full output out. Sharding: 8 cores = 4 batches x 2 head-groups.
Each core handles one batch and 4 of the 8 base heads: even cores heads
{0,1,4,5}, odd cores {2,3,6,7}. Heads 0-3 ("heavy") have effective head size
128; heads 4-7 ("light") have effective head size 64 (their mixed weight rows
64:128 are exactly zero), so the two light heads are packed into one 128-wide
tensor for projections and run as two attention units with half-zeroed K.

Per-core pipeline (Tile-scheduled, tj = 512-wide t-chunk):
  - Weight mixing patterns effA/effB via tiny rank-1 matmuls; W = base * eff
    (DVE); W^T via DMA-xbar transpose (no PE).
  - x: DRAM -> SBUF (f32), cast bf16 on GpSimd, x^T via DMA-xbar transpose.
  - Projections per tj: q^T,k^T per unit (PE, 4x512 matmuls); v for all 3
    units batched in one N=384 matmul per 128-t tile.
  - Attention per tj, 4 units interleaved so PE alternates scores (feeding
    ACT exp) and AV (consuming previous unit's probs): scoresT [s,t] in
    2-bank PSUM groups, exp on ACT (scale folded, no max-subtraction:
    |scaled scores| < 3), causal diag masking via tri-mask multiply on
    GpSimd, AV with ones-column-fused row sums, normalize on DVE.
  - Output assembled in [128,512] o-tiles, one DMA store per 128-t tile.
"""
import sys

for p in ("/opt/trn_rl_repo",):
    if p not in sys.path:
        sys.path.append(p)

import numpy as np

import concourse.bass as bass
import concourse.tile as tile
from concourse import bacc, mybir
from concourse.bass_utils import run_bass_kernel_spmd

FP32 = mybir.dt.float32
BF16 = mybir.dt.bfloat16
AF = mybir.ActivationFunctionType
ALU = mybir.AluOpType

T = 2048
C = 512
HS = 128          # heavy head size (= padded head size)
NT128 = T // 128  # 16
NT512 = T // 512  # 4
NCC = C // 128    # 4
SCALE = float(1.0 / np.sqrt(128.0))
SGRP = 3          # score chunks (512 wide) per exp group
VW = 394          # v tile: [h0 128+ones @0][h1 128+ones @132][l0 64+ones @264][l1 64+ones @329]

# (kt idx, qt idx, v_lo, v_hi, out col); w = v_hi-v_lo includes the ones col
ATT = [
    (0, 0, 0, 129, 0),
    (1, 1, 132, 261, 128),
    (2, 2, 264, 329, 256),
    (3, 2, 329, 394, 384),
]

_CACHE = {}


def _build():
    nc = bacc.Bacc("TRN2", target_bir_lowering=False, debug=False, num_devices=8)
    x_d = nc.dram_tensor("x", [T, C], FP32, kind="ExternalInput")
    w_d = nc.dram_tensor("w", [4, 1], FP32, kind="ExternalInput")
    bq_d = nc.dram_tensor("bq", [4, HS, C], FP32, kind="ExternalInput")
    bk_d = nc.dram_tensor("bk", [4, HS, C], FP32, kind="ExternalInput")
    bv_d = nc.dram_tensor("bv", [4, HS, C], FP32, kind="ExternalInput")
    out_d = nc.dram_tensor("out", [T, 4 * HS], FP32, kind="ExternalOutput")

    with tile.TileContext(nc) as tc:
        _emit(nc, tc, x_d, w_d, bq_d, bk_d, bv_d, out_d)
    nc.compile()
    return nc


def _emit(nc, tc, x_d, w_d, bq_d, bk_d, bv_d, out_d):
    from contextlib import ExitStack

    ctx = ExitStack()
    prep_ctx = ExitStack()
    with ctx:
        # ---- persistent SBUF pools ----
        const_p = ctx.enter_context(tc.tile_pool(name="const", bufs=1))
        wts_p = ctx.enter_context(tc.tile_pool(name="wts", bufs=1))
        xt_p = ctx.enter_context(tc.tile_pool(name="xt", bufs=1))
        xst_p = ctx.enter_context(tc.tile_pool(name="xst", bufs=2))
        qk_p = ctx.enter_context(tc.tile_pool(name="qk", bufs=1))
        v_p = ctx.enter_context(tc.tile_pool(name="v", bufs=1))
        pt_p = ctx.enter_context(tc.tile_pool(name="pt", bufs=1))
        o_p = ctx.enter_context(tc.tile_pool(name="o", bufs=2))
        r_p = ctx.enter_context(tc.tile_pool(name="r", bufs=6))
        # ---- PSUM pools: 3 x 2-bank score groups + 2 x 1-bank small = 8
        sps = ctx.enter_context(tc.tile_pool(name="sps", bufs=2, space="PSUM"))
        ps = ctx.enter_context(tc.tile_pool(name="ps", bufs=2, space="PSUM"))
        stage_p = prep_ctx.enter_context(tc.tile_pool(name="stage", bufs=3))

        # ================= constants =================
        ones_b = const_p.tile([128, 128], BF16, tag="ones_b")
        nc.vector.memset(ones_b[:], 1.0)
        ident_b = const_p.tile([128, 128], BF16, tag="ident_b")
        nc.gpsimd.affine_select(
            ident_b[:], ones_b[:], pattern=[[1, 128]],
            compare_op=ALU.is_equal, fill=0.0, base=0, channel_multiplier=-1)
        # causal triangle for the diagonal 128x128 block: tri[s, t] = (t >= s)
        tri = const_p.tile([128, 128], BF16, tag="tri")
        nc.gpsimd.affine_select(
            tri[:], ones_b[:], pattern=[[1, 128]],
            compare_op=ALU.is_ge, fill=0.0, base=0, channel_multiplier=-1)
        ones_f = const_p.tile([128, C], FP32, tag="ones_f")
        nc.vector.memset(ones_f[:], 1.0)

        # ================= eff patterns via region fills =================
        # effA[d, e] = w0(d<64)(e<256) + w1(d<32)(e<256) + w2 + w3(d<64)
        # effB[d, e] = w1(d%64<32)(e<256) + w3   (packed light, d%64)
        # Piecewise constant -> fill regions with weight sums. Sums via a
        # tiny K=4 matmul broadcasting (w * SMAT) across 128 partitions.
        w_row = const_p.tile([1, 4], FP32, tag="w_row")
        nc.sync.dma_start(w_row[:], w_d.ap().rearrange("a b -> b a"))
        # cols: r0=w0+w1+w2+w3  r1=w0+w2+w3  r2=w2  r3=w2+w3  r4=w1+w3  r5=w3
        esr = const_p.tile([1, 8], FP32, tag="esr")
        nc.vector.tensor_add(esr[:, 3:4], w_row[:, 2:3], w_row[:, 3:4])
        nc.vector.tensor_add(esr[:, 1:2], w_row[:, 0:1], esr[:, 3:4])
        nc.vector.tensor_add(esr[:, 0:1], w_row[:, 1:2], esr[:, 1:2])
        nc.vector.tensor_add(esr[:, 4:5], w_row[:, 1:2], w_row[:, 3:4])
        nc.vector.tensor_copy(esr[:, 2:3], w_row[:, 2:3])
        nc.vector.tensor_copy(esr[:, 5:6], w_row[:, 3:4])
        esr_b = const_p.tile([1, 8], BF16, tag="esr_b")
        nc.vector.tensor_copy(esr_b[:], esr[:])
        ones1 = const_p.tile([1, 128], BF16, tag="ones1")
        nc.vector.memset(ones1[:], 1.0)
        esum_ps = ps.tile([128, 512], FP32, tag="ps")
        nc.tensor.matmul(esum_ps[:, 0:8], ones1[:], esr_b[:], start=True, stop=True)
        esum = const_p.tile([128, 8], FP32, tag="esum")
        nc.vector.tensor_copy(esum[:], esum_ps[:, 0:8])
        effA = const_p.tile([128, C], FP32, tag="effA")
        effB = const_p.tile([128, C], FP32, tag="effB")
        for eff, rows, cols, r in (
                (effA, slice(0, 32), slice(0, 256), 0),
                (effA, slice(32, 64), slice(0, 256), 1),
                (effA, slice(64, 128), slice(0, 256), 2),
                (effA, slice(0, 64), slice(256, 512), 3),
                (effA, slice(64, 128), slice(256, 512), 2),
                (effB, slice(0, 32), slice(0, 256), 4),
                (effB, slice(32, 64), slice(0, 256), 5),
                (effB, slice(64, 96), slice(0, 256), 4),
                (effB, slice(96, 128), slice(0, 256), 5),
                (effB, slice(0, 128), slice(256, 512), 5)):
            nc.vector.tensor_scalar_mul(
                eff[rows, cols], ones_f[rows, cols], esum[rows, r:r + 1])

        # ======= effective weights, transposed on PE (idle in prologue) ==
        # wqt/wkt[hj]: [c128, (cc, d128)]; wvt: [c128, (cc, 3*128 d)] for v.
        wqt = [wts_p.tile([128, 512], BF16, name=f"wqt{j}", tag=f"wqt{j}")
               for j in range(3)]
        wkt = [wts_p.tile([128, 512], BF16, name=f"wkt{j}", tag=f"wkt{j}")
               for j in range(3)]
        wvt = wts_p.tile([128, NCC * 384], BF16, tag="wvt")
        wvt3 = wvt[:].rearrange("p (a d) -> p a d", a=NCC)

        def emit_wprep(hj):
            for pi, bd in enumerate((bq_d, bk_d, bv_d)):
                w_bf = stage_p.tile([128, C], BF16, tag="w_bf", bufs=4)
                base = stage_p.tile([128, C], FP32, tag="base", bufs=9)
                eng = nc.vector if pi % 2 == 0 else nc.gpsimd
                if hj < 2:
                    nc.scalar.dma_start(base[:], bd.ap()[hj])
                    eng.tensor_mul(w_bf[:], base[:], effA[:])
                else:
                    nc.scalar.dma_start(base[0:64, :], bd.ap()[2][0:64, :])
                    nc.scalar.dma_start(base[64:128, :], bd.ap()[3][0:64, :])
                    eng.tensor_mul(w_bf[:], base[:], effB[:])
                pt = ps.tile([128, 512], BF16, name="pw", tag="ps")
                for cc in range(NCC):
                    nc.tensor.transpose(
                        pt[:, cc * 128:(cc + 1) * 128],
                        w_bf[:, cc * 128:(cc + 1) * 128], ident_b[:])
                if pi == 0:
                    nc.vector.tensor_copy(wqt[hj][:], pt[:])
                elif pi == 1:
                    nc.vector.tensor_copy(wkt[hj][:], pt[:])
                else:
                    nc.vector.tensor_copy(
                        wvt3[:, :, hj * 128:(hj + 1) * 128],
                        pt[:].rearrange("p (a d) -> p a d", a=NCC))

        # ================= x -> bf16 -> x^T via DMA xbar =================
        xt_all = xt_p.tile([128, NCC * T], BF16, tag="xt_all")
        xt = [xt_all[:, cc * T:(cc + 1) * T] for cc in range(NCC)]
        xt3 = xt_all[:].rearrange("p (a t) -> p a t", a=NCC)

        def emit_x(tj, use_pe=False):
            for h in range(2):
                base_row = tj * 512 + h * 256
                xs = xst_p.tile([128, 1024], FP32, name="xs", tag="xs", bufs=3)
                xq = nc.scalar if use_pe else nc.sync
                xq.dma_start(
                    xs[:].rearrange("p (a c) -> p a c", a=2),
                    x_d.ap()[base_row:base_row + 256, :].rearrange(
                        "(a p) c -> p a c", a=2))
                xb = xst_p.tile([128, 1024], BF16, name="xb", tag="xb", bufs=3)
                nc.vector.tensor_copy(xb[:], xs[:])
                for r in range(2):
                    tt = tj * 4 + h * 2 + r
                    if use_pe:
                        pt = ps.tile([128, 512], BF16, name="px", tag="ps")
                        for cc in range(NCC):
                            nc.tensor.transpose(
                                pt[:, cc * 128:(cc + 1) * 128],
                                xb[:, r * 512 + cc * 128:r * 512 + (cc + 1) * 128],
                                ident_b[:])
                        nc.vector.tensor_copy(
                            xt3[:, :, tt * 128:(tt + 1) * 128],
                            pt[:].rearrange("p (a t) -> p a t", a=NCC))
                    else:
                        nc.sync.dma_start_transpose(
                            xt3[:, :, tt * 128:(tt + 1) * 128],
                            xb[:, r * 512:(r + 1) * 512])

        # ================= projections =================
        qt = [qk_p.tile([128, T], BF16, name=f"qt{h}", tag=f"qt{h}") for h in range(3)]
        # kt: heavy0, heavy1, l0 (rows 64:128 zero), l1 (rows 0:64 zero).
        kt = [qk_p.tile([128, T], BF16, name=f"kt{h}", tag=f"kt{h}") for h in range(4)]
        vt = [v_p.tile([128, VW], BF16, name=f"v{i}", tag=f"v{i}")
              for i in range(NT128)]

        def emit_qk_proj(hj, tj):
            sl = slice(tj * 512, (tj + 1) * 512)
            for wtt, is_q in ((wqt[hj], True), (wkt[hj], False)):
                p = ps.tile([128, 512], FP32, name="p", tag="ps")
                for cc in range(NCC):
                    nc.tensor.matmul(
                        p[:], wtt[:, cc * 128:(cc + 1) * 128],
                        xt[cc][:, sl],
                        start=(cc == 0), stop=(cc == NCC - 1))
                if is_q:
                    nc.vector.tensor_copy(qt[hj][:, sl], p[:])
                elif hj < 2:
                    nc.vector.tensor_copy(kt[hj][:, sl], p[:])
                else:
                    nc.vector.tensor_copy(kt[2][0:64, sl], p[0:64, :])
                    nc.vector.tensor_copy(kt[3][64:128, sl], p[64:128, :])

        def emit_v_proj(i):
            p = ps.tile([128, 512], FP32, name="p", tag="ps")
            for cc in range(NCC):
                nc.tensor.matmul(
                    p[:, 0:384], xt[cc][:, i * 128:(i + 1) * 128],
                    wvt3[:, cc, :],
                    start=(cc == 0), stop=(cc == NCC - 1))
            vt_i = vt[i]
            heavy = vt_i[:, 0:264].rearrange("p (a w) -> p a w", a=2)
            nc.vector.tensor_copy(
                heavy[:, :, 0:128],
                p[:, 0:256].rearrange("p (a w) -> p a w", a=2))
            light = vt_i[:, 264:394].rearrange("p (a w) -> p a w", a=2)
            nc.vector.tensor_copy(
                light[:, :, 0:64],
                p[:, 256:384].rearrange("p (a w) -> p a w", a=2))
            nc.gpsimd.memset(heavy[:, :, 128:129], 1.0)
            nc.gpsimd.memset(light[:, :, 64:65], 1.0)

        def emit_proj(tj):
            for hj in range(3):
                emit_qk_proj(hj, tj)
            for i in range(4 * tj, 4 * tj + 4):
                emit_v_proj(i)

        # ================= attention =================
        def emit_scores(units, tj, av_work):
            """Emit scores+exp for one heavy unit ([u]) or both light units
            ([2, 3], whose K=64 matmuls run concurrently in disjoint PE row
            groups); interleave pending AV m-chunk emissions between score
            groups so PE has ready work while ACT drains exp."""
            S = 4 * tj + 4
            pts = [pt_p.tile([128, NT128 * 512], BF16,
                             name="pt", tag="pt", bufs=3) for _ in units]
            g = 0
            while g < S:
                gw = min(SGRP, S - g)
                sps_t = [sps.tile([128, SGRP * 512], FP32, name="sp", tag="sps")
                         for _ in units]
                for k in range(gw):
                    i = g + k
                    for n, u in enumerate(units):
                        (ktj, qtj, _, _, _) = ATT[u]
                        nc.tensor.matmul(
                            sps_t[n][:, k * 512:(k + 1) * 512],
                            kt[ktj][:, i * 128:(i + 1) * 128],
                            qt[qtj][:, tj * 512:(tj + 1) * 512],
                            start=True, stop=True)
                for n, u in enumerate(units):
                    nc.scalar.activation(
                        pts[n][:, g * 512:(g + gw) * 512],
                        sps_t[n][:, 0:gw * 512], AF.Exp, scale=SCALE)
                    # tri-mask diag chunks as soon as their exp is emitted
                    for k in range(gw):
                        i = g + k
                        r = i - 4 * tj
                        if r >= 0:
                            blk = slice(i * 512 + r * 128,
                                        i * 512 + (r + 1) * 128)
                            nc.gpsimd.tensor_mul(
                                pts[n][:, blk], pts[n][:, blk], tri[:])
                g += gw
                if g < S and av_work:
                    av_work.pop(0)()
            for f in av_work:
                f()
            del av_work[:]
            return pts

        def emit_av_m(u, tj, pt, otiles, m):
            (ktj, qtj, v_lo, v_hi, ocol) = ATT[u]
            w = v_hi - v_lo
            ti = 4 * tj + m
            if u == 0:
                ot = o_p.tile([128, 512], FP32, name=f"o{m}", tag=f"o{m}")
                otiles[m] = ot
                # zero the upper halves of the light-head columns
                nc.gpsimd.memset(
                    ot[:, 256:512].rearrange(
                        "p (a w) -> p a w", a=2)[:, :, 64:128], 0.0)
            ot = otiles[m]
            op = ps.tile([128, 512], FP32, name="op", tag="ps")
            for i in range(ti + 1):
                nc.tensor.matmul(
                    op[:, 0:w],
                    pt[:, i * 512 + m * 128: i * 512 + (m + 1) * 128],
                    vt[i][:, v_lo:v_hi],
                    start=(i == 0), stop=(i == ti))
            rec = r_p.tile([128, 1], FP32, name="rec", tag="rec")
            nc.vector.reciprocal(rec[:], op[:, w - 1:w])
            nc.vector.tensor_scalar_mul(
                ot[:, ocol:ocol + (w - 1)], op[:, 0:w - 1], rec[:])

        def av_ms(u, tj, pt, otiles):
            return [
                (lambda m=m: emit_av_m(u, tj, pt, otiles, m)) for m in range(4)]

        def emit_stores(tj, otiles):
            for m in range(4):
                ti = 4 * tj + m
                nc.sync.dma_start(
                    out_d.ap()[ti * 128:(ti + 1) * 128, :], otiles[m][:])

        # ================= schedule =================
        emit_x(0, use_pe=True)
        for hj in range(3):
            emit_wprep(hj)
        nc.gpsimd.memset(kt[2][64:128, :], 0.0)
        nc.gpsimd.memset(kt[3][0:64, :], 0.0)
        emit_proj(0)
        prep_ctx.close()

        emit_x(1)
        for tj in range(NT512):
            otiles = {}
            pt0 = emit_scores([0], tj, [])[0]
            pt1 = emit_scores([1], tj, av_ms(0, tj, pt0, otiles))[0]
            pt2 = emit_scores([2], tj, av_ms(1, tj, pt1, otiles))[0]
            if tj + 2 < NT512:
                emit_x(tj + 2)
            work3 = av_ms(2, tj, pt2, otiles)
            if tj + 1 < NT512:
                if tj < 2:
                    # small tj: get next projections (and their DVE copies)
                    # queued early so the tj boundary doesn't stall on them
                    emit_proj(tj + 1)
                else:
                    for hj in range(3):
                        work3.append(lambda hj=hj: emit_qk_proj(hj, tj + 1))
                    for i in range(4 * (tj + 1), 4 * (tj + 1) + 4):
                        work3.append(lambda i=i: emit_v_proj(i))
            pt3 = emit_scores([3], tj, work3)[0]
            for f in av_ms(3, tj, pt3, otiles):
                f()
            emit_stores(tj, otiles)


def _shard_inputs(x, weights, base_K, base_Q, base_V):
    in_maps = []
    for c in range(8):
        b = c // 2
        hsel = [0, 1, 4, 5] if c % 2 == 0 else [2, 3, 6, 7]
        in_maps.append({
            "x": np.ascontiguousarray(x[b]),
            "w": np.ascontiguousarray(weights.reshape(4, 1)),
            "bq": np.ascontiguousarray(base_Q[hsel]),
            "bk": np.ascontiguousarray(base_K[hsel]),
            "bv": np.ascontiguousarray(base_V[hsel]),
        })
    return in_maps


def _gather(results):
    out = np.zeros((4, T, 8 * HS), np.float32)
    for c in range(8):
        o = results[c]["out"]
        hsel = [0, 1, 4, 5] if c % 2 == 0 else [2, 3, 6, 7]
        for j, h in enumerate(hsel):
            out[c // 2][:, h * HS:(h + 1) * HS] = o[:, j * HS:(j + 1) * HS]
    return out


def get_nc():
    if "nc" not in _CACHE:
        _CACHE["nc"] = _build()
    return _CACHE["nc"]


def kernel(x, weights, base_K, base_Q, base_V):
    x = np.asarray(x, np.float32)
    weights = np.asarray(weights, np.float32)
    base_K = np.asarray(base_K, np.float32)
    base_Q = np.asarray(base_Q, np.float32)
    base_V = np.asarray(base_V, np.float32)
    nc = get_nc()
    in_maps = _shard_inputs(x, weights, base_K, base_Q, base_V)
    res = run_bass_kernel_spmd(nc, in_maps, core_ids=list(range(8)))
    return _gather(res.results)


# revision 33
# speedup vs baseline: 1.0026x; 1.0026x over previous
"""Trainium2 Bass kernel for nn_MixedHeadsV2 (mixed-head causal attention).

Full inputs in, full output out. Sharding: 8 cores = 4 batches x 2 head-groups.
Each core handles one batch and 4 of the 8 base heads: even cores heads
{0,1,4,5}, odd cores {2,3,6,7}. Heads 0-3 ("heavy") have effective head size
128; heads 4-7 ("light") have effective head size 64 (their mixed weight rows
64:128 are exactly zero), so the two light heads are packed into one 128-wide
tensor for projections and run as two attention units with half-zeroed K.

Per-core pipeline (Tile-scheduled, tj = 512-wide t-chunk):
  - Weight mixing patterns effA/effB via tiny rank-1 matmuls; W = base * eff
    (DVE); W^T via DMA-xbar transpose (no PE).
  - x: DRAM -> SBUF (f32), cast bf16 on GpSimd, x^T via DMA-xbar transpose.
  - Projections per tj: q^T,k^T per unit (PE, 4x512 matmuls); v for all 3
    units batched in one N=384 matmul per 128-t tile.
  - Attention per tj, 4 units interleaved so PE alternates scores (feeding
    ACT exp) and AV (consuming previous unit's probs): scoresT [s,t] in
    2-bank PSUM groups, exp on ACT (scale folded, no max-subtraction:
    |scaled scores| < 3), causal diag masking via tri-mask multiply on
    GpSimd, AV with ones-column-fused row sums, normalize on DVE.
  - Output assembled in [128,512] o-tiles, one DMA store per 128-t tile.
"""
import sys

for p in ("/opt/trn_rl_repo",):
    if p not in sys.path:
        sys.path.append(p)

import numpy as np

import concourse.bass as bass
import concourse.tile as tile
from concourse import bacc, mybir
from concourse.bass_utils import run_bass_kernel_spmd

FP32 = mybir.dt.float32
BF16 = mybir.dt.bfloat16
AF = mybir.ActivationFunctionType
ALU = mybir.AluOpType

T = 2048
C = 512
HS = 128          # heavy head size (= padded head size)
NT128 = T // 128  # 16
NT512 = T // 512  # 4
NCC = C // 128    # 4
SCALE = float(1.0 / np.sqrt(128.0))
SGRP = 2          # score chunks (512 wide) per exp group
VW = 394          # v tile: [h0 128+ones @0][h1 128+ones @132][l0 64+ones @264][l1 64+ones @329]

# (kt idx, qt idx, v_lo, v_hi, out col); w = v_hi-v_lo includes the ones col
ATT = [
    (0, 0, 0, 129, 0),
    (1, 1, 132, 261, 128),
    (2, 2, 264, 329, 256),
    (3, 2, 329, 394, 384),
]

_CACHE = {}


def _build():
    nc = bacc.Bacc("TRN2", target_bir_lowering=False, debug=False, num_devices=8)
    x_d = nc.dram_tensor("x", [T, C], FP32, kind="ExternalInput")
    w_d = nc.dram_tensor("w", [4, 1], FP32, kind="ExternalInput")
    bq_d = nc.dram_tensor("bq", [4, HS, C], FP32, kind="ExternalInput")
    bk_d = nc.dram_tensor("bk", [4, HS, C], FP32, kind="ExternalInput")
    bv_d = nc.dram_tensor("bv", [4, HS, C], FP32, kind="ExternalInput")
    out_d = nc.dram_tensor("out", [T, 4 * HS], FP32, kind="ExternalOutput")

    with tile.TileContext(nc) as tc:
        _emit(nc, tc, x_d, w_d, bq_d, bk_d, bv_d, out_d)
    nc.compile()
    return nc


def _emit(nc, tc, x_d, w_d, bq_d, bk_d, bv_d, out_d):
    from contextlib import ExitStack

    ctx = ExitStack()
    prep_ctx = ExitStack()
    with ctx:
        # ---- persistent SBUF pools ----
        const_p = ctx.enter_context(tc.tile_pool(name="const", bufs=1))
        wts_p = ctx.enter_context(tc.tile_pool(name="wts", bufs=1))
        xt_p = ctx.enter_context(tc.tile_pool(name="xt", bufs=1))
        xst_p = ctx.enter_context(tc.tile_pool(name="xst", bufs=2))
        qk_p = ctx.enter_context(tc.tile_pool(name="qk", bufs=1))
        v_p = ctx.enter_context(tc.tile_pool(name="v", bufs=1))
        pt_p = ctx.enter_context(tc.tile_pool(name="pt", bufs=1))
        o_p = ctx.enter_context(tc.tile_pool(name="o", bufs=2))
        r_p = ctx.enter_context(tc.tile_pool(name="r", bufs=6))
        # ---- PSUM pools: 3 x 2-bank score groups + 2 x 1-bank small = 8
        sps = ctx.enter_context(tc.tile_pool(name="sps", bufs=2, space="PSUM"))
        ps = ctx.enter_context(tc.tile_pool(name="ps", bufs=4, space="PSUM"))
        stage_p = prep_ctx.enter_context(tc.tile_pool(name="stage", bufs=3))

        # ================= constants =================
        ones_b = const_p.tile([128, 128], BF16, tag="ones_b")
        nc.vector.memset(ones_b[:], 1.0)
        ident_b = const_p.tile([128, 128], BF16, tag="ident_b")
        nc.gpsimd.affine_select(
            ident_b[:], ones_b[:], pattern=[[1, 128]],
            compare_op=ALU.is_equal, fill=0.0, base=0, channel_multiplier=-1)
        # causal triangle for the diagonal 128x128 block: tri[s, t] = (t >= s)
        tri = const_p.tile([128, 128], BF16, tag="tri")
        nc.gpsimd.affine_select(
            tri[:], ones_b[:], pattern=[[1, 128]],
            compare_op=ALU.is_ge, fill=0.0, base=0, channel_multiplier=-1)
        ones_f = const_p.tile([128, C], FP32, tag="ones_f")
        nc.vector.memset(ones_f[:], 1.0)

        # ================= eff patterns via region fills =================
        # effA[d, e] = w0(d<64)(e<256) + w1(d<32)(e<256) + w2 + w3(d<64)
        # effB[d, e] = w1(d%64<32)(e<256) + w3   (packed light, d%64)
        # Piecewise constant -> fill regions with weight sums. Sums via a
        # tiny K=4 matmul broadcasting (w * SMAT) across 128 partitions.
        w_row = const_p.tile([1, 4], FP32, tag="w_row")
        nc.sync.dma_start(w_row[:], w_d.ap().rearrange("a b -> b a"))
        # cols: r0=w0+w1+w2+w3  r1=w0+w2+w3  r2=w2  r3=w2+w3  r4=w1+w3  r5=w3
        esr = const_p.tile([1, 8], FP32, tag="esr")
        nc.vector.tensor_add(esr[:, 3:4], w_row[:, 2:3], w_row[:, 3:4])
        nc.vector.tensor_add(esr[:, 1:2], w_row[:, 0:1], esr[:, 3:4])
        nc.vector.tensor_add(esr[:, 0:1], w_row[:, 1:2], esr[:, 1:2])
        nc.vector.tensor_add(esr[:, 4:5], w_row[:, 1:2], w_row[:, 3:4])
        nc.vector.tensor_copy(esr[:, 2:3], w_row[:, 2:3])
        nc.vector.tensor_copy(esr[:, 5:6], w_row[:, 3:4])
        esr_b = const_p.tile([1, 8], BF16, tag="esr_b")
        nc.vector.tensor_copy(esr_b[:], esr[:])
        ones1 = const_p.tile([1, 128], BF16, tag="ones1")
        nc.vector.memset(ones1[:], 1.0)
        esum_ps = ps.tile([128, 512], FP32, tag="ps")
        nc.tensor.matmul(esum_ps[:, 0:8], ones1[:], esr_b[:], start=True, stop=True)
        esum = const_p.tile([128, 8], FP32, tag="esum")
        nc.vector.tensor_copy(esum[:], esum_ps[:, 0:8])
        effA = const_p.tile([128, C], FP32, tag="effA")
        effB = const_p.tile([128, C], FP32, tag="effB")
        for eff, rows, cols, r in (
                (effA, slice(0, 32), slice(0, 256), 0),
                (effA, slice(32, 64), slice(0, 256), 1),
                (effA, slice(64, 128), slice(0, 256), 2),
                (effA, slice(0, 64), slice(256, 512), 3),
                (effA, slice(64, 128), slice(256, 512), 2),
                (effB, slice(0, 32), slice(0, 256), 4),
                (effB, slice(32, 64), slice(0, 256), 5),
                (effB, slice(64, 96), slice(0, 256), 4),
                (effB, slice(96, 128), slice(0, 256), 5),
                (effB, slice(0, 128), slice(256, 512), 5)):
            nc.vector.tensor_scalar_mul(
                eff[rows, cols], ones_f[rows, cols], esum[rows, r:r + 1])

        # ======= effective weights, transposed on PE (idle in prologue) ==
        # wqt/wkt[hj]: [c128, (cc, d128)]; wvt: [c128, (cc, 3*128 d)] for v.
        wqt = [wts_p.tile([128, 512], BF16, name=f"wqt{j}", tag=f"wqt{j}")
               for j in range(3)]
        wkt = [wts_p.tile([128, 512], BF16, name=f"wkt{j}", tag=f"wkt{j}")
               for j in range(3)]
        wvt = wts_p.tile([128, NCC * 384], BF16, tag="wvt")
        wvt3 = wvt[:].rearrange("p (a d) -> p a d", a=NCC)

        def emit_wprep(hj):
            for pi, bd in enumerate((bq_d, bk_d, bv_d)):
                w_bf = stage_p.tile([128, C], BF16, tag="w_bf", bufs=4)
                base = stage_p.tile([128, C], FP32, tag="base", bufs=9)
                eng = nc.vector if pi % 2 == 0 else nc.gpsimd
                if hj < 2:
                    nc.scalar.dma_start(base[:], bd.ap()[hj])
                    eng.tensor_mul(w_bf[:], base[:], effA[:])
                else:
                    nc.scalar.dma_start(base[0:64, :], bd.ap()[2][0:64, :])
                    nc.scalar.dma_start(base[64:128, :], bd.ap()[3][0:64, :])
                    eng.tensor_mul(w_bf[:], base[:], effB[:])
                pt = ps.tile([128, 512], BF16, name="pw", tag="ps")
                for cc in range(NCC):
                    nc.tensor.transpose(
                        pt[:, cc * 128:(cc + 1) * 128],
                        w_bf[:, cc * 128:(cc + 1) * 128], ident_b[:])
                if pi == 0:
                    nc.vector.tensor_copy(wqt[hj][:], pt[:])
                elif pi == 1:
                    nc.vector.tensor_copy(wkt[hj][:], pt[:])
                else:
                    nc.vector.tensor_copy(
                        wvt3[:, :, hj * 128:(hj + 1) * 128],
                        pt[:].rearrange("p (a d) -> p a d", a=NCC))

        # ================= x -> bf16 -> x^T via DMA xbar =================
        xt_all = xt_p.tile([128, NCC * T], BF16, tag="xt_all")
        xt = [xt_all[:, cc * T:(cc + 1) * T] for cc in range(NCC)]
        xt3 = xt_all[:].rearrange("p (a t) -> p a t", a=NCC)

        def emit_x(tj, use_pe=False):
            for h in range(2):
                base_row = tj * 512 + h * 256
                xs = xst_p.tile([128, 1024], FP32, name="xs", tag="xs", bufs=3)
                xq = nc.scalar if use_pe else nc.sync
                xq.dma_start(
                    xs[:].rearrange("p (a c) -> p a c", a=2),
                    x_d.ap()[base_row:base_row + 256, :].rearrange(
                        "(a p) c -> p a c", a=2))
                xb = xst_p.tile([128, 1024], BF16, name="xb", tag="xb", bufs=3)
                nc.vector.tensor_copy(xb[:], xs[:])
                for r in range(2):
                    tt = tj * 4 + h * 2 + r
                    if use_pe:
                        pt = ps.tile([128, 512], BF16, name="px", tag="ps")
                        for cc in range(NCC):
                            nc.tensor.transpose(
                                pt[:, cc * 128:(cc + 1) * 128],
                                xb[:, r * 512 + cc * 128:r * 512 + (cc + 1) * 128],
                                ident_b[:])
                        nc.vector.tensor_copy(
                            xt3[:, :, tt * 128:(tt + 1) * 128],
                            pt[:].rearrange("p (a t) -> p a t", a=NCC))
                    else:
                        nc.sync.dma_start_transpose(
                            xt3[:, :, tt * 128:(tt + 1) * 128],
                            xb[:, r * 512:(r + 1) * 512])

        # ================= projections =================
        qt = [qk_p.tile([128, T], BF16, name=f"qt{h}", tag=f"qt{h}") for h in range(3)]
        # kt: heavy0, heavy1, l0 (rows 64:128 zero), l1 (rows 0:64 zero).
        kt = [qk_p.tile([128, T], BF16, name=f"kt{h}", tag=f"kt{h}") for h in range(4)]
        vt = [v_p.tile([128, VW], BF16, name=f"v{i}", tag=f"v{i}")
              for i in range(NT128)]

        def emit_qk_proj(hj, tj):
            sl = slice(tj * 512, (tj + 1) * 512)
            for wtt, is_q in ((wqt[hj], True), (wkt[hj], False)):
                p = ps.tile([128, 512], FP32, name="p", tag="ps")
                for cc in range(NCC):
                    nc.tensor.matmul(
                        p[:], wtt[:, cc * 128:(cc + 1) * 128],
                        xt[cc][:, sl],
                        start=(cc == 0), stop=(cc == NCC - 1))
                if is_q:
                    nc.vector.tensor_copy(qt[hj][:, sl], p[:])
                elif hj < 2:
                    nc.vector.tensor_copy(kt[hj][:, sl], p[:])
                else:
                    nc.vector.tensor_copy(kt[2][0:64, sl], p[0:64, :])
                    nc.vector.tensor_copy(kt[3][64:128, sl], p[64:128, :])

        def emit_v_proj(i):
            p = ps.tile([128, 512], FP32, name="p", tag="ps")
            for cc in range(NCC):
                nc.tensor.matmul(
                    p[:, 0:384], xt[cc][:, i * 128:(i + 1) * 128],
                    wvt3[:, cc, :],
                    start=(cc == 0), stop=(cc == NCC - 1))
            vt_i = vt[i]
            heavy = vt_i[:, 0:264].rearrange("p (a w) -> p a w", a=2)
            nc.vector.tensor_copy(
                heavy[:, :, 0:128],
                p[:, 0:256].rearrange("p (a w) -> p a w", a=2))
            light = vt_i[:, 264:394].rearrange("p (a w) -> p a w", a=2)
            nc.vector.tensor_copy(
                light[:, :, 0:64],
                p[:, 256:384].rearrange("p (a w) -> p a w", a=2))
            nc.gpsimd.memset(heavy[:, :, 128:129], 1.0)
            nc.gpsimd.memset(light[:, :, 64:65], 1.0)

        def emit_proj(tj):
            for hj in range(3):
                emit_qk_proj(hj, tj)
            for i in range(4 * tj, 4 * tj + 4):
                emit_v_proj(i)

        # ================= attention =================
        def emit_scores(units, tj, av_work):
            """Emit scores+exp for one heavy unit ([u]) or both light units
            ([2, 3], whose K=64 matmuls run concurrently in disjoint PE row
            groups); interleave pending AV m-chunk emissions between score
            groups so PE has ready work while ACT drains exp."""
            S = 4 * tj + 4
            pts = [pt_p.tile([128, NT128 * 512], BF16,
                             name="pt", tag="pt", bufs=3) for _ in units]
            g = 0
            while g < S:
                gw = min(SGRP, S - g)
                sps_t = [sps.tile([128, SGRP * 512], FP32, name="sp", tag="sps")
                         for _ in units]
                for k in range(gw):
                    i = g + k
                    for n, u in enumerate(units):
                        (ktj, qtj, _, _, _) = ATT[u]
                        nc.tensor.matmul(
                            sps_t[n][:, k * 512:(k + 1) * 512],
                            kt[ktj][:, i * 128:(i + 1) * 128],
                            qt[qtj][:, tj * 512:(tj + 1) * 512],
                            start=True, stop=True)
                for n, u in enumerate(units):
                    nc.scalar.activation(
                        pts[n][:, g * 512:(g + gw) * 512],
                        sps_t[n][:, 0:gw * 512], AF.Exp, scale=SCALE)
                    # tri-mask diag chunks as soon as their exp is emitted
                    for k in range(gw):
                        i = g + k
                        r = i - 4 * tj
                        if r >= 0:
                            blk = slice(i * 512 + r * 128,
                                        i * 512 + (r + 1) * 128)
                            nc.gpsimd.tensor_mul(
                                pts[n][:, blk], pts[n][:, blk], tri[:])
                g += gw
                if g < S and av_work:
                    av_work.pop(0)()
            for f in av_work:
                f()
            del av_work[:]
            return pts

        def emit_av_m(u, tj, pt, otiles, m):
            (ktj, qtj, v_lo, v_hi, ocol) = ATT[u]
            w = v_hi - v_lo
            ti = 4 * tj + m
            if u == 0:
                ot = o_p.tile([128, 512], FP32, name=f"o{m}", tag=f"o{m}")
                otiles[m] = ot
                # zero the upper halves of the light-head columns
                nc.gpsimd.memset(
                    ot[:, 256:512].rearrange(
                        "p (a w) -> p a w", a=2)[:, :, 64:128], 0.0)
            ot = otiles[m]
            op = ps.tile([128, 512], FP32, name="op", tag="ps")
            for i in range(ti + 1):
                nc.tensor.matmul(
                    op[:, 0:w],
                    pt[:, i * 512 + m * 128: i * 512 + (m + 1) * 128],
                    vt[i][:, v_lo:v_hi],
                    start=(i == 0), stop=(i == ti))
            rec = r_p.tile([128, 1], FP32, name="rec", tag="rec")
            nc.vector.reciprocal(rec[:], op[:, w - 1:w])
            nc.vector.tensor_scalar_mul(
                ot[:, ocol:ocol + (w - 1)], op[:, 0:w - 1], rec[:])

        def av_ms(u, tj, pt, otiles):
            return [
                (lambda m=m: emit_av_m(u, tj, pt, otiles, m)) for m in range(4)]

        def emit_stores(tj, otiles):
            for m in range(4):
                ti = 4 * tj + m
                nc.sync.dma_start(
                    out_d.ap()[ti * 128:(ti + 1) * 128, :], otiles[m][:])

        # ================= schedule =================
        emit_x(0, use_pe=True)
        for hj in range(3):
            emit_wprep(hj)
        nc.gpsimd.memset(kt[2][64:128, :], 0.0)
        nc.gpsimd.memset(kt[3][0:64, :], 0.0)
        emit_proj(0)
        prep_ctx.close()

        emit_x(1)
        carry = []
        for tj in range(NT512):
            otiles = {}
            pt0 = emit_scores([0], tj, carry)[0]
            pt1 = emit_scores([1], tj, av_ms(0, tj, pt0, otiles))[0]
            pt2 = emit_scores([2], tj, av_ms(1, tj, pt1, otiles))[0]
            if tj + 2 < NT512:
                emit_x(tj + 2)
            work3 = av_ms(2, tj, pt2, otiles)
            if tj + 1 < NT512:
                if tj < 2:
                    # small tj: get next projections (and their DVE copies)
                    # queued early so the tj boundary doesn't stall on them
                    emit_proj(tj + 1)
                else:
                    for hj in range(3):
                        work3.append(lambda hj=hj: emit_qk_proj(hj, tj + 1))
                    for i in range(4 * (tj + 1), 4 * (tj + 1) + 4):
                        work3.append(lambda i=i: emit_v_proj(i))
            pt3 = emit_scores([3], tj, work3)[0]
            # defer AV(3) + stores into the next tj's first score pass so
            # the tj boundary has no serial tail
            carry = av_ms(3, tj, pt3, otiles)
            carry.append(lambda tj=tj, ot=otiles: emit_stores(tj, ot))
        for f in carry:
            f()


def _shard_inputs(x, weights, base_K, base_Q, base_V):
    in_maps = []
    for c in range(8):
        b = c // 2
        hsel = [0, 1, 4, 5] if c % 2 == 0 else [2, 3, 6, 7]
        in_maps.append({
            "x": np.ascontiguousarray(x[b]),
            "w": np.ascontiguousarray(weights.reshape(4, 1)),
            "bq": np.ascontiguousarray(base_Q[hsel]),
            "bk": np.ascontiguousarray(base_K[hsel]),
            "bv": np.ascontiguousarray(base_V[hsel]),
        })
    return in_maps


def _gather(results):
    out = np.zeros((4, T, 8 * HS), np.float32)
    for c in range(8):
        o = results[c]["out"]
        hsel = [0, 1, 4, 5] if c % 2 == 0 else [2, 3, 6, 7]
        for j, h in enumerate(hsel):
            out[c // 2][:, h * HS:(h + 1) * HS] = o[:, j * HS:(j + 1) * HS]
    return out


def get_nc():
    if "nc" not in _CACHE:
        _CACHE["nc"] = _build()
    return _CACHE["nc"]


def kernel(x, weights, base_K, base_Q, base_V):
    x = np.asarray(x, np.float32)
    weights = np.asarray(weights, np.float32)
    base_K = np.asarray(base_K, np.float32)
    base_Q = np.asarray(base_Q, np.float32)
    base_V = np.asarray(base_V, np.float32)
    nc = get_nc()
    in_maps = _shard_inputs(x, weights, base_K, base_Q, base_V)
    res = run_bass_kernel_spmd(nc, in_maps, core_ids=list(range(8)))
    return _gather(res.results)


# revision 34
# speedup vs baseline: 1.0152x; 1.0127x over previous
"""Trainium2 Bass kernel for nn_MixedHeadsV2 (mixed-head causal attention).

Full inputs in, full output out. Sharding: 8 cores = 4 batches x 2 head-groups.
Each core handles one batch and 4 of the 8 base heads: even cores heads
{0,1,4,5}, odd cores {2,3,6,7}. Heads 0-3 ("heavy") have effective head size
128; heads 4-7 ("light") have effective head size 64 (their mixed weight rows
64:128 are exactly zero), so the two light heads are packed into one 128-wide
tensor for projections and run as two attention units with half-zeroed K.

Per-core pipeline (Tile-scheduled, tj = 512-wide t-chunk):
  - Weight mixing patterns effA/effB via tiny rank-1 matmuls; W = base * eff
    (DVE); W^T via DMA-xbar transpose (no PE).
  - x: DRAM -> SBUF (f32), cast bf16 on GpSimd, x^T via DMA-xbar transpose.
  - Projections per tj: q^T,k^T per unit (PE, 4x512 matmuls); v for all 3
    units batched in one N=384 matmul per 128-t tile.
  - Attention per tj, 4 units interleaved so PE alternates scores (feeding
    ACT exp) and AV (consuming previous unit's probs): scoresT [s,t] in
    2-bank PSUM groups, exp on ACT (scale folded, no max-subtraction:
    |scaled scores| < 3), causal diag masking via tri-mask multiply on
    GpSimd, AV with ones-column-fused row sums, normalize on DVE.
  - Output assembled in [128,512] o-tiles, one DMA store per 128-t tile.
"""
import sys

for p in ("/opt/trn_rl_repo",):
    if p not in sys.path:
        sys.path.append(p)

import numpy as np

import concourse.bass as bass
import concourse.tile as tile
from concourse import bacc, mybir
from concourse.bass_utils import run_bass_kernel_spmd

FP32 = mybir.dt.float32
BF16 = mybir.dt.bfloat16
AF = mybir.ActivationFunctionType
ALU = mybir.AluOpType

T = 2048
C = 512
HS = 128          # heavy head size (= padded head size)
NT128 = T // 128  # 16
NT512 = T // 512  # 4
NCC = C // 128    # 4
SCALE = float(1.0 / np.sqrt(128.0))
SGRP = 2          # score chunks (512 wide) per exp group
VW = 394          # v tile: [h0 128+ones @0][h1 128+ones @132][l0 64+ones @264][l1 64+ones @329]

# (kt idx, qt idx, v_lo, v_hi, out col); w = v_hi-v_lo includes the ones col
ATT = [
    (0, 0, 0, 129, 0),
    (1, 1, 132, 261, 128),
    (2, 2, 264, 329, 256),
    (3, 2, 329, 394, 384),
]

_CACHE = {}


def _build():
    nc = bacc.Bacc("TRN2", target_bir_lowering=False, debug=False, num_devices=8)
    x_d = nc.dram_tensor("x", [T, C], FP32, kind="ExternalInput")
    w_d = nc.dram_tensor("w", [4, 1], FP32, kind="ExternalInput")
    bq_d = nc.dram_tensor("bq", [4, HS, C], FP32, kind="ExternalInput")
    bk_d = nc.dram_tensor("bk", [4, HS, C], FP32, kind="ExternalInput")
    bv_d = nc.dram_tensor("bv", [4, HS, C], FP32, kind="ExternalInput")
    out_d = nc.dram_tensor("out", [T, 4 * HS], FP32, kind="ExternalOutput")

    with tile.TileContext(nc) as tc:
        _emit(nc, tc, x_d, w_d, bq_d, bk_d, bv_d, out_d)
    nc.compile()
    return nc


def _emit(nc, tc, x_d, w_d, bq_d, bk_d, bv_d, out_d):
    from contextlib import ExitStack

    ctx = ExitStack()
    prep_ctx = ExitStack()
    with ctx:
        # ---- persistent SBUF pools ----
        const_p = ctx.enter_context(tc.tile_pool(name="const", bufs=1))
        wts_p = ctx.enter_context(tc.tile_pool(name="wts", bufs=1))
        xt_p = ctx.enter_context(tc.tile_pool(name="xt", bufs=1))
        xst_p = ctx.enter_context(tc.tile_pool(name="xst", bufs=2))
        qk_p = ctx.enter_context(tc.tile_pool(name="qk", bufs=1))
        v_p = ctx.enter_context(tc.tile_pool(name="v", bufs=1))
        pt_p = ctx.enter_context(tc.tile_pool(name="pt", bufs=1))
        o_p = ctx.enter_context(tc.tile_pool(name="o", bufs=2))
        r_p = ctx.enter_context(tc.tile_pool(name="r", bufs=6))
        # ---- PSUM pools: 3 x 2-bank score groups + 2 x 1-bank small = 8
        sps = ctx.enter_context(tc.tile_pool(name="sps", bufs=2, space="PSUM"))
        ps = ctx.enter_context(tc.tile_pool(name="ps", bufs=4, space="PSUM"))
        stage_p = prep_ctx.enter_context(tc.tile_pool(name="stage", bufs=3))

        # ================= constants =================
        ones_b = const_p.tile([128, 128], BF16, tag="ones_b")
        nc.vector.memset(ones_b[:], 1.0)
        ident_b = const_p.tile([128, 128], BF16, tag="ident_b")
        nc.gpsimd.affine_select(
            ident_b[:], ones_b[:], pattern=[[1, 128]],
            compare_op=ALU.is_equal, fill=0.0, base=0, channel_multiplier=-1)
        # causal triangle for the diagonal 128x128 block: tri[s, t] = (t >= s)
        tri = const_p.tile([128, 128], BF16, tag="tri")
        nc.gpsimd.affine_select(
            tri[:], ones_b[:], pattern=[[1, 128]],
            compare_op=ALU.is_ge, fill=0.0, base=0, channel_multiplier=-1)
        ones_f = const_p.tile([128, C], FP32, tag="ones_f")
        nc.vector.memset(ones_f[:], 1.0)

        # ================= eff patterns via region fills =================
        # effA[d, e] = w0(d<64)(e<256) + w1(d<32)(e<256) + w2 + w3(d<64)
        # effB[d, e] = w1(d%64<32)(e<256) + w3   (packed light, d%64)
        # Piecewise constant -> fill regions with weight sums. Sums via a
        # tiny K=4 matmul broadcasting (w * SMAT) across 128 partitions.
        w_row = const_p.tile([1, 4], FP32, tag="w_row")
        nc.sync.dma_start(w_row[:], w_d.ap().rearrange("a b -> b a"))
        # cols: r0=w0+w1+w2+w3  r1=w0+w2+w3  r2=w2  r3=w2+w3  r4=w1+w3  r5=w3
        esr = const_p.tile([1, 8], FP32, tag="esr")
        nc.vector.tensor_add(esr[:, 3:4], w_row[:, 2:3], w_row[:, 3:4])
        nc.vector.tensor_add(esr[:, 1:2], w_row[:, 0:1], esr[:, 3:4])
        nc.vector.tensor_add(esr[:, 0:1], w_row[:, 1:2], esr[:, 1:2])
        nc.vector.tensor_add(esr[:, 4:5], w_row[:, 1:2], w_row[:, 3:4])
        nc.vector.tensor_copy(esr[:, 2:3], w_row[:, 2:3])
        nc.vector.tensor_copy(esr[:, 5:6], w_row[:, 3:4])
        esr_b = const_p.tile([1, 8], BF16, tag="esr_b")
        nc.vector.tensor_copy(esr_b[:], esr[:])
        ones1 = const_p.tile([1, 128], BF16, tag="ones1")
        nc.vector.memset(ones1[:], 1.0)
        esum_ps = ps.tile([128, 512], FP32, tag="ps")
        nc.tensor.matmul(esum_ps[:, 0:8], ones1[:], esr_b[:], start=True, stop=True)
        esum = const_p.tile([128, 8], FP32, tag="esum")
        nc.vector.tensor_copy(esum[:], esum_ps[:, 0:8])
        effA = const_p.tile([128, C], FP32, tag="effA")
        effB = const_p.tile([128, C], FP32, tag="effB")
        for eff, rows, cols, r in (
                (effA, slice(0, 32), slice(0, 256), 0),
                (effA, slice(32, 64), slice(0, 256), 1),
                (effA, slice(64, 128), slice(0, 256), 2),
                (effA, slice(0, 64), slice(256, 512), 3),
                (effA, slice(64, 128), slice(256, 512), 2),
                (effB, slice(0, 32), slice(0, 256), 4),
                (effB, slice(32, 64), slice(0, 256), 5),
                (effB, slice(64, 96), slice(0, 256), 4),
                (effB, slice(96, 128), slice(0, 256), 5),
                (effB, slice(0, 128), slice(256, 512), 5)):
            nc.vector.tensor_scalar_mul(
                eff[rows, cols], ones_f[rows, cols], esum[rows, r:r + 1])

        # ======= effective weights, transposed on PE (idle in prologue) ==
        # wqt/wkt[hj]: [c128, (cc, d128)]; wvt: [c128, (cc, 3*128 d)] for v.
        wqt = [wts_p.tile([128, 512], BF16, name=f"wqt{j}", tag=f"wqt{j}")
               for j in range(3)]
        wkt = [wts_p.tile([128, 512], BF16, name=f"wkt{j}", tag=f"wkt{j}")
               for j in range(3)]
        wvt = wts_p.tile([128, NCC * 384], BF16, tag="wvt")
        wvt3 = wvt[:].rearrange("p (a d) -> p a d", a=NCC)

        def emit_wprep(hj):
            for pi, bd in enumerate((bq_d, bk_d, bv_d)):
                w_bf = stage_p.tile([128, C], BF16, tag="w_bf", bufs=4)
                base = stage_p.tile([128, C], FP32, tag="base", bufs=9)
                eng = nc.vector if pi % 2 == 0 else nc.gpsimd
                if hj < 2:
                    nc.scalar.dma_start(base[:], bd.ap()[hj])
                    eng.tensor_mul(w_bf[:], base[:], effA[:])
                else:
                    nc.scalar.dma_start(base[0:64, :], bd.ap()[2][0:64, :])
                    nc.scalar.dma_start(base[64:128, :], bd.ap()[3][0:64, :])
                    eng.tensor_mul(w_bf[:], base[:], effB[:])
                pt = ps.tile([128, 512], BF16, name="pw", tag="ps")
                for cc in range(NCC):
                    nc.tensor.transpose(
                        pt[:, cc * 128:(cc + 1) * 128],
                        w_bf[:, cc * 128:(cc + 1) * 128], ident_b[:])
                if pi == 0:
                    nc.vector.tensor_copy(wqt[hj][:], pt[:])
                elif pi == 1:
                    nc.vector.tensor_copy(wkt[hj][:], pt[:])
                else:
                    nc.vector.tensor_copy(
                        wvt3[:, :, hj * 128:(hj + 1) * 128],
                        pt[:].rearrange("p (a d) -> p a d", a=NCC))

        # ================= x -> bf16 -> x^T via DMA xbar =================
        xt_all = xt_p.tile([128, NCC * T], BF16, tag="xt_all")
        xt = [xt_all[:, cc * T:(cc + 1) * T] for cc in range(NCC)]
        xt3 = xt_all[:].rearrange("p (a t) -> p a t", a=NCC)

        def emit_x(tj, use_pe=False):
            for h in range(2):
                base_row = tj * 512 + h * 256
                xs = xst_p.tile([128, 1024], FP32, name="xs", tag="xs", bufs=3)
                xq = nc.scalar if use_pe else nc.sync
                xq.dma_start(
                    xs[:].rearrange("p (a c) -> p a c", a=2),
                    x_d.ap()[base_row:base_row + 256, :].rearrange(
                        "(a p) c -> p a c", a=2))
                xb = xst_p.tile([128, 1024], BF16, name="xb", tag="xb", bufs=3)
                nc.vector.tensor_copy(xb[:], xs[:])
                for r in range(2):
                    tt = tj * 4 + h * 2 + r
                    if use_pe:
                        pt = ps.tile([128, 512], BF16, name="px", tag="ps")
                        for cc in range(NCC):
                            nc.tensor.transpose(
                                pt[:, cc * 128:(cc + 1) * 128],
                                xb[:, r * 512 + cc * 128:r * 512 + (cc + 1) * 128],
                                ident_b[:])
                        nc.vector.tensor_copy(
                            xt3[:, :, tt * 128:(tt + 1) * 128],
                            pt[:].rearrange("p (a t) -> p a t", a=NCC))
                    else:
                        nc.sync.dma_start_transpose(
                            xt3[:, :, tt * 128:(tt + 1) * 128],
                            xb[:, r * 512:(r + 1) * 512])

        # ================= projections =================
        qt = [qk_p.tile([128, T], BF16, name=f"qt{h}", tag=f"qt{h}") for h in range(3)]
        # kt: heavy0, heavy1, l0 (rows 64:128 zero), l1 (rows 0:64 zero).
        kt = [qk_p.tile([128, T], BF16, name=f"kt{h}", tag=f"kt{h}") for h in range(4)]
        vt = [v_p.tile([128, VW], BF16, name=f"v{i}", tag=f"v{i}")
              for i in range(NT128)]

        def emit_qk_proj(hj, tj):
            sl = slice(tj * 512, (tj + 1) * 512)
            for wtt, is_q in ((wqt[hj], True), (wkt[hj], False)):
                p = ps.tile([128, 512], FP32, name="p", tag="ps")
                for cc in range(NCC):
                    nc.tensor.matmul(
                        p[:], wtt[:, cc * 128:(cc + 1) * 128],
                        xt[cc][:, sl],
                        start=(cc == 0), stop=(cc == NCC - 1))
                if is_q:
                    nc.vector.tensor_copy(qt[hj][:, sl], p[:])
                elif hj < 2:
                    nc.vector.tensor_copy(kt[hj][:, sl], p[:])
                else:
                    nc.vector.tensor_copy(kt[2][0:64, sl], p[0:64, :])
                    nc.vector.tensor_copy(kt[3][64:128, sl], p[64:128, :])

        def emit_v_proj(i):
            p = ps.tile([128, 512], FP32, name="p", tag="ps")
            for cc in range(NCC):
                nc.tensor.matmul(
                    p[:, 0:384], xt[cc][:, i * 128:(i + 1) * 128],
                    wvt3[:, cc, :],
                    start=(cc == 0), stop=(cc == NCC - 1))
            vt_i = vt[i]
            heavy = vt_i[:, 0:264].rearrange("p (a w) -> p a w", a=2)
            nc.vector.tensor_copy(
                heavy[:, :, 0:128],
                p[:, 0:256].rearrange("p (a w) -> p a w", a=2))
            light = vt_i[:, 264:394].rearrange("p (a w) -> p a w", a=2)
            nc.vector.tensor_copy(
                light[:, :, 0:64],
                p[:, 256:384].rearrange("p (a w) -> p a w", a=2))
            nc.gpsimd.memset(heavy[:, :, 128:129], 1.0)
            nc.gpsimd.memset(light[:, :, 64:65], 1.0)

        def emit_proj(tj):
            for hj in range(3):
                emit_qk_proj(hj, tj)
            for i in range(4 * tj, 4 * tj + 4):
                emit_v_proj(i)

        # ================= attention =================
        def emit_scores(units, tj, av_work):
            """Emit scores+exp for one heavy unit ([u]) or both light units
            ([2, 3], whose K=64 matmuls run concurrently in disjoint PE row
            groups); interleave pending AV m-chunk emissions between score
            groups so PE has ready work while ACT drains exp."""
            S = 4 * tj + 4
            pts = [pt_p.tile([128, NT128 * 512], BF16,
                             name="pt", tag="pt", bufs=3) for _ in units]
            g = 0
            while g < S:
                gw = min(SGRP, S - g)
                sps_t = [sps.tile([128, SGRP * 512], FP32, name="sp", tag="sps")
                         for _ in units]
                for k in range(gw):
                    i = g + k
                    # diag superblock: t-cols below 128r are never read by
                    # AV (strictly upper-triangle) -> skip computing them
                    off = max(i - 4 * tj, 0) * 128
                    for n, u in enumerate(units):
                        (ktj, qtj, _, _, _) = ATT[u]
                        nc.tensor.matmul(
                            sps_t[n][:, k * 512 + off:(k + 1) * 512],
                            kt[ktj][:, i * 128:(i + 1) * 128],
                            qt[qtj][:, tj * 512 + off:(tj + 1) * 512],
                            start=True, stop=True)
                for n, u in enumerate(units):
                    nc.scalar.activation(
                        pts[n][:, g * 512:(g + gw) * 512],
                        sps_t[n][:, 0:gw * 512], AF.Exp, scale=SCALE)
                    # tri-mask diag chunks as soon as their exp is emitted
                    for k in range(gw):
                        i = g + k
                        r = i - 4 * tj
                        if r >= 0:
                            blk = slice(i * 512 + r * 128,
                                        i * 512 + (r + 1) * 128)
                            nc.gpsimd.tensor_mul(
                                pts[n][:, blk], pts[n][:, blk], tri[:])
                g += gw
                if g < S and av_work:
                    av_work.pop(0)()
            for f in av_work:
                f()
            del av_work[:]
            return pts

        def emit_av_m(u, tj, pt, otiles, m):
            (ktj, qtj, v_lo, v_hi, ocol) = ATT[u]
            w = v_hi - v_lo
            ti = 4 * tj + m
            if u == 0:
                ot = o_p.tile([128, 512], FP32, name=f"o{m}", tag=f"o{m}")
                otiles[m] = ot
                # zero the upper halves of the light-head columns
                nc.gpsimd.memset(
                    ot[:, 256:512].rearrange(
                        "p (a w) -> p a w", a=2)[:, :, 64:128], 0.0)
            ot = otiles[m]
            op = ps.tile([128, 512], FP32, name="op", tag="ps")
            for i in range(ti + 1):
                nc.tensor.matmul(
                    op[:, 0:w],
                    pt[:, i * 512 + m * 128: i * 512 + (m + 1) * 128],
                    vt[i][:, v_lo:v_hi],
                    start=(i == 0), stop=(i == ti))
            rec = r_p.tile([128, 1], FP32, name="rec", tag="rec")
            nc.vector.reciprocal(rec[:], op[:, w - 1:w])
            nc.vector.tensor_scalar_mul(
                ot[:, ocol:ocol + (w - 1)], op[:, 0:w - 1], rec[:])

        def av_ms(u, tj, pt, otiles):
            return [
                (lambda m=m: emit_av_m(u, tj, pt, otiles, m)) for m in range(4)]

        def emit_stores(tj, otiles):
            for m in range(4):
                ti = 4 * tj + m
                nc.sync.dma_start(
                    out_d.ap()[ti * 128:(ti + 1) * 128, :], otiles[m][:])

        # ================= schedule =================
        emit_x(0, use_pe=True)
        for hj in range(3):
            emit_wprep(hj)
        nc.gpsimd.memset(kt[2][64:128, :], 0.0)
        nc.gpsimd.memset(kt[3][0:64, :], 0.0)
        emit_proj(0)
        prep_ctx.close()

        emit_x(1)
        carry = []
        for tj in range(NT512):
            otiles = {}
            pt0 = emit_scores([0], tj, carry)[0]
            pt1 = emit_scores([1], tj, av_ms(0, tj, pt0, otiles))[0]
            pt2 = emit_scores([2], tj, av_ms(1, tj, pt1, otiles))[0]
            if tj + 2 < NT512:
                emit_x(tj + 2)
            work3 = av_ms(2, tj, pt2, otiles)
            if tj + 1 < NT512:
                if tj < 2:
                    # small tj: get next projections (and their DVE copies)
                    # queued early so the tj boundary doesn't stall on them
                    emit_proj(tj + 1)
                else:
                    for hj in range(3):
                        work3.append(lambda hj=hj: emit_qk_proj(hj, tj + 1))
                    for i in range(4 * (tj + 1), 4 * (tj + 1) + 4):
                        work3.append(lambda i=i: emit_v_proj(i))
            pt3 = emit_scores([3], tj, work3)[0]
            # defer AV(3) + stores into the next tj's first score pass so
            # the tj boundary has no serial tail
            carry = av_ms(3, tj, pt3, otiles)
            carry.append(lambda tj=tj, ot=otiles: emit_stores(tj, ot))
        for f in carry:
            f()


def _shard_inputs(x, weights, base_K, base_Q, base_V):
    in_maps = []
    for c in range(8):
        b = c // 2
        hsel = [0, 1, 4, 5] if c % 2 == 0 else [2, 3, 6, 7]
        in_maps.append({
            "x": np.ascontiguousarray(x[b]),
            "w": np.ascontiguousarray(weights.reshape(4, 1)),
            "bq": np.ascontiguousarray(base_Q[hsel]),
            "bk": np.ascontiguousarray(base_K[hsel]),
            "bv": np.ascontiguousarray(base_V[hsel]),
        })
    return in_maps


def _gather(results):
    out = np.zeros((4, T, 8 * HS), np.float32)
    for c in range(8):
        o = results[c]["out"]
        hsel = [0, 1, 4, 5] if c % 2 == 0 else [2, 3, 6, 7]
        for j, h in enumerate(hsel):
            out[c // 2][:, h * HS:(h + 1) * HS] = o[:, j * HS:(j + 1) * HS]
    return out


def get_nc():
    if "nc" not in _CACHE:
        _CACHE["nc"] = _build()
    return _CACHE["nc"]


def kernel(x, weights, base_K, base_Q, base_V):
    x = np.asarray(x, np.float32)
    weights = np.asarray(weights, np.float32)
    base_K = np.asarray(base_K, np.float32)
    base_Q = np.asarray(base_Q, np.float32)
    base_V = np.asarray(base_V, np.float32)
    nc = get_nc()
    in_maps = _shard_inputs(x, weights, base_K, base_Q, base_V)
    res = run_bass_kernel_spmd(nc, in_maps, core_ids=list(range(8)))
    return _gather(res.results)
